# revision 11
# baseline (speedup 1.0000x reference)
"""Trainium2 Bass kernel for single-head causal attention.

Transposed-softmax layout (S^T, no PE transposes, no max-subtraction),
interleaved-key causal skip, host flash-combine, host-side weight
fusion Mh = WK @ WQ^T / sqrt(d), and a fully-resident fp16/bf16 SBUF
plan:

  score path (Xq, Mh, Xk, K~) in fp16  -- 1.0 PE cycle/row like f32r,
      but half the DMA bytes / SBUF footprint and FWL weight loads
  softmax slab, V, ohat in bf16        -- exp() needs bf16's exponent
      range (scores reach ~+/-50, exp up to ~1e22)
  all matmul accumulation in f32 PSUM; host combines
      out = (ohat0 + ohat1) / (l0 + l1) per batch in f32.

Everything is resident in SBUF (~180 KB/partition of 208), so every
input tile is DMA'd exactly once with no write-after-read hazards:
the PE stream never waits on a buffer-reuse semaphore.  Per-core
phases: warmup (bridges DMA cold start + HAM clock ramp) -> K~ =
Mh @ Xk^T -> g3 score block -> V projection -> attention groups
largest-first.
"""

import numpy as np

import concourse.bass as bass
from concourse import bacc
import concourse.mybir as mybir
import concourse.tile as tile
from concourse.bass_utils import run_bass_kernel_spmd

P = 128
B, S, DIN, DOUT = 4, 2048, 1024, 1024
KSH = S // 2        # key rows per core
KO = DIN // P       # 8 contraction sub-tiles
NT = KSH // P       # 8 key tiles per core
QG = 512            # query-group width (psum free dim)
NG = S // QG        # 4 query groups
F32 = mybir.dt.float32
F16 = mybir.dt.float16
BF16 = mybir.dt.bfloat16
WARM = 8            # warmup matmuls (bridge DMA cold start + HAM ramp)

_NC_CACHE = {}


def _build_bass():
    nc = bacc.Bacc()
    xqT = nc.declare_dram_parameter("xqT", [DIN, S], F16, isOutput=False)
    xkT = nc.declare_dram_parameter("xkT", [DIN, KSH], F16, isOutput=False)
    xvT = nc.declare_dram_parameter("xvT", [DIN, KSH], F16, isOutput=False)
    mhT = nc.declare_dram_parameter("mhT", [P, KO * KO * P], F16, isOutput=False)
    wv = nc.declare_dram_parameter("wv", [P, KO * DOUT], F16, isOutput=False)
    maskT = nc.declare_dram_parameter("maskT", [P, QG], BF16, isOutput=False)
    ohat = nc.declare_dram_parameter("ohat", [S, DOUT], BF16, isOutput=True)
    l_out = nc.declare_dram_parameter("l", [1, S], F32, isOutput=True)

    xq3 = xqT[:, :].rearrange("(o p) q -> p o q", p=P)
    xk3 = xkT[:, :].rearrange("(o p) s -> p o s", p=P)
    xv3 = xvT[:, :].rearrange("(o p) s -> p o s", p=P)
    # host pre-swizzled: mh4[p, t, k, ii] = Mh.T[k*128+p, t*128+ii] so each
    # dout-slice load is one contiguous 2KB line per partition
    mh4 = mhT[:, :].rearrange("p (t k x) -> p t k x", t=KO, k=KO)
    # wv host layout IS the sbuf layout [p][k][dout]: straight big-line loads
    wv3 = wv[:, :].rearrange("p (k x) -> p k x", k=KO)

    with tile.TileContext(nc) as tc:
        with tc.tile_pool(name="persist", bufs=1) as pp:
            # ---- PE warmup: keep the tensor engine busy through the DMA
            # cold start so the HAM clock ramp (3.4us busy window) completes
            # early.  The warm tile lives in the persist pool: a scoped pool
            # would be reused by the input tiles below, making their DMAs
            # wait (WAR) for the warmup matmuls.
            with tc.tile_pool(name="ps_w", bufs=1, space="PSUM") as pswarm:
                wsc = pp.tile([P, 512], F16, name="warm_sc")
                nc.vector.memset(wsc, 0.0)
                wps = pswarm.tile([P, 512], F32, name="warm_ps")
                for i in range(WARM):
                    nc.tensor.matmul(
                        wps, lhsT=wsc[:, 0:P], rhs=wsc,
                        start=(i == 0), stop=(i == WARM - 1),
                    )
            # all inputs resident; every tile DMA'd exactly once
            mh_sb = pp.tile([P, KO, DIN], F16, name="mh")
            xk_sb = pp.tile([P, KO, KSH], F16, name="xk")
            xv_sb = pp.tile([P, KO, KSH], F16, name="xv")
            xq_sb = pp.tile([P, KO, S], F16, name="xq")
            wv_sb = pp.tile([P, KO, DOUT], F16, name="wv")
            kqt_sb = pp.tile([P, KO, KSH], F16, name="kqt")
            v_sb = pp.tile([P, NT, DOUT], BF16, name="v")
            slab_first = pp.tile([P, NT, QG], BF16, name="expT_first")
            m0_sb = pp.tile([P, QG], BF16, name="mask0")
            zeros_sb = pp.tile([P, QG // 2], BF16, name="zeros")
            ones_sb = pp.tile([P, 1], BF16, name="ones")
            l_sb = pp.tile([1, S], F32, name="l_row")
            nc.vector.memset(zeros_sb, 0.0)
            nc.vector.memset(ones_sb, 1.0)

            # ---- DMA issue order == first-use order.  All targets are
            # fresh resident tiles, so no descriptor ever waits on compute.
            # The K~ chunk-0 feed goes on the scalar queue: the scalar
            # engine reaches its first instruction ~1us before sync does,
            # and the two queues then issue descriptors in parallel.
            nc.scalar.dma_start(out=mh_sb[:, :, 0:P], in_=mh4[:, 0])
            for o in range(0, KO, 2):   # K~ chunk 0 feed, o-pair granularity
                nc.scalar.dma_start(
                    out=xk_sb[:, o : o + 2, 0:QG], in_=xk3[:, o : o + 2, 0:QG]
                )
            for s in range(1, KO):      # rest of Mh, slice-wise
                nc.sync.dma_start(
                    out=mh_sb[:, :, s * P : (s + 1) * P], in_=mh4[:, s]
                )
            for o in range(0, KO, 2):   # K~ chunk 1 feed
                nc.sync.dma_start(
                    out=xk_sb[:, o : o + 2, QG:KSH], in_=xk3[:, o : o + 2, QG:KSH]
                )
            g3 = (NG - 1) * QG          # g3 score block inputs
            for o in range(0, KO, 4):
                nc.sync.dma_start(
                    out=xq_sb[:, o : o + 4, g3 : g3 + QG],
                    in_=xq3[:, o : o + 4, g3 : g3 + QG],
                )
            nc.sync.dma_start(out=m0_sb, in_=maskT[:, :])
            for k in range(0, KO, 4):   # V-phase inputs
                nc.sync.dma_start(
                    out=wv_sb[:, k : k + 4, :], in_=wv3[:, k : k + 4, :]
                )
            for o in range(0, KO, 4):
                nc.sync.dma_start(
                    out=xv_sb[:, o : o + 4, :], in_=xv3[:, o : o + 4, :]
                )
            for g in (2, 1, 0):         # remaining query groups, use order
                for o in range(0, KO, 4):
                    nc.sync.dma_start(
                        out=xq_sb[:, o : o + 4, g * QG : (g + 1) * QG],
                        in_=xq3[:, o : o + 4, g * QG : (g + 1) * QG],
                    )

            # ---- Phase K~: K~^T = Mh @ Xk^T  [din, keys]
            with tc.tile_pool(name="ps_k", bufs=4, space="PSUM") as psK:
                for c in range(2):
                    for o in range(KO):
                        ps = psK.tile([P, QG], F32, name="kq_ps")
                        for k in range(KO):
                            nc.tensor.matmul(
                                ps,
                                lhsT=mh_sb[:, k, o * P : (o + 1) * P],
                                rhs=xk_sb[:, k, c * QG : (c + 1) * QG],
                                start=(k == 0),
                                stop=(k == KO - 1),
                            )
                        nc.vector.tensor_copy(
                            kqt_sb[:, o, c * QG : (c + 1) * QG], ps
                        )

            with (
                tc.tile_pool(name="exp_tmp", bufs=2) as epool,
                tc.tile_pool(name="slab", bufs=2) as slabpool,
                tc.tile_pool(name="ao", bufs=3) as aopool,
                tc.tile_pool(name="ps_s", bufs=3, space="PSUM") as psS,
                tc.tile_pool(name="ps_l", bufs=1, space="PSUM") as psL,
            ):
                H = QG // 2

                def score_chunk(slab, g, kt):
                    """Scores+exp for one (group, k-tile) [128, 512] chunk of
                    S^T.  kt == 2g+1's first 256 queries are fully masked for
                    both cores (interleaved-key geometry): zero-fill and
                    compute only the second half.  The causal mask pattern is
                    group-independent, so one resident m0 tile serves every
                    diagonal chunk; masking is a post-exp 0/1 multiply so
                    bf16 rounding never touches raw logits."""
                    q0 = g * QG
                    if kt == 2 * g + 1:
                        ps = psS.tile([P, QG], F32, name="score_ps")
                        ph = ps[:, H:]
                        for io in range(KO):
                            nc.tensor.matmul(
                                ph,
                                lhsT=kqt_sb[:, io, kt * P : (kt + 1) * P],
                                rhs=xq_sb[:, io, q0 + H : q0 + QG],
                                start=(io == 0),
                                stop=(io == KO - 1),
                            )
                        nc.vector.tensor_copy(slab[:, kt, :H], zeros_sb)
                        et = epool.tile([P, QG], BF16, name="exp_tmp")
                        nc.scalar.activation(
                            et[:, :H], ph, mybir.ActivationFunctionType.Exp
                        )
                        nc.vector.tensor_tensor(
                            slab[:, kt, H:], et[:, :H], m0_sb[:, :H],
                            mybir.AluOpType.mult,
                        )
                        return
                    ps = psS.tile([P, QG], F32, name="score_ps")
                    for io in range(KO):
                        nc.tensor.matmul(
                            ps,
                            lhsT=kqt_sb[:, io, kt * P : (kt + 1) * P],
                            rhs=xq_sb[:, io, q0 : q0 + QG],
                            start=(io == 0),
                            stop=(io == KO - 1),
                        )
                    if kt == 2 * g:
                        et = epool.tile([P, QG], BF16, name="exp_tmp")
                        nc.scalar.activation(
                            et, ps, mybir.ActivationFunctionType.Exp
                        )
                        nc.vector.tensor_tensor(
                            slab[:, kt, :], et, m0_sb, mybir.AluOpType.mult
                        )
                    else:
                        nc.scalar.activation(
                            slab[:, kt, :], ps, mybir.ActivationFunctionType.Exp
                        )

                # ---- g3 score block: needs only kqt + xq_g3, runs while the
                # V-phase inputs finish streaming
                for kt in range(NT):
                    score_chunk(slab_first, NG - 1, kt)

                # ---- Phase V: V = Xv @ Wv for this core's key blocks
                with tc.tile_pool(name="ps_v", bufs=4, space="PSUM") as psV:
                    for t in range(NT):
                        for dh in range(2):
                            ps = psV.tile([P, QG], F32, name="v_ps")
                            for k in range(KO):
                                nc.tensor.matmul(
                                    ps,
                                    lhsT=xv_sb[:, k, t * P : (t + 1) * P],
                                    rhs=wv_sb[:, k, dh * QG : (dh + 1) * QG],
                                    start=(k == 0),
                                    stop=(k == KO - 1),
                                )
                            nc.vector.tensor_copy(
                                v_sb[:, t, dh * QG : (dh + 1) * QG], ps
                            )

                # ---- Phase A: causal-skip transposed-softmax attention,
                # largest group first
                psO_cm = tc.tile_pool(name="ps_o", bufs=4, space="PSUM")
                psO = psO_cm.__enter__()
                for g in reversed(range(NG)):
                    lim = min(NT, 2 * g + 2)   # k-tiles actually attended
                    if g == NG - 1:
                        slab = slab_first
                    else:
                        slab = slabpool.tile([P, NT, QG], BF16, name="expT")
                        for kt in range(lim):
                            score_chunk(slab, g, kt)

                    ps_l = psL.tile([1, QG], F32, name="l_ps")
                    for kt in range(lim):
                        nc.tensor.matmul(
                            ps_l,
                            lhsT=ones_sb,
                            rhs=slab[:, kt, :],
                            start=(kt == 0),
                            stop=(kt == lim - 1),
                        )
                    nc.vector.tensor_copy(l_sb[:, g * QG : (g + 1) * QG], ps_l)
                    nc.sync.dma_start(
                        out=l_out[:, g * QG : (g + 1) * QG],
                        in_=l_sb[:, g * QG : (g + 1) * QG],
                    )

                    for t in range(QG // P):
                        # first 256 queries of the group can't see the last
                        # (fully masked) key tile
                        kts = range(lim - 1) if t < 2 else range(lim)
                        o_sb = aopool.tile([P, DOUT], BF16, name="attn_out")
                        q0 = g * QG + t * P
                        for dh in range(2):
                            ps = psO.tile([P, QG], F32, name="out_ps")
                            for kt in kts:
                                nc.tensor.matmul(
                                    ps,
                                    lhsT=slab[:, kt, t * P : (t + 1) * P],
                                    rhs=v_sb[:, kt, dh * QG : (dh + 1) * QG],
                                    start=(kt == kts[0]),
                                    stop=(kt == kts[-1]),
                                )
                            if dh == 0:
                                nc.scalar.copy(o_sb[:, :QG], ps)
                            else:
                                nc.vector.tensor_copy(o_sb[:, QG:], ps)
                        # alternate store queues: descriptor generation is
                        # ~0.6us serial per queue, which otherwise backs up
                        # the tail after the last matmul
                        eng = nc.sync if t % 2 == 0 else nc.scalar
                        eng.dma_start(out=ohat[q0 : q0 + P, :], in_=o_sb)
                psO_cm.__exit__(None, None, None)
    nc.finalize()
    return nc


def _get_nc():
    if "nc" not in _NC_CACHE:
        _NC_CACHE["nc"] = _build_bass()
    return _NC_CACHE["nc"]


def _key_index(hk):
    """Global key rows owned by core hk: interleaved 128-row blocks."""
    blocks = np.arange(hk, S // P, 2)
    return (blocks[:, None] * P + np.arange(P)[None, :]).reshape(-1)


def _mask_tile(hk):
    """Multiplicative causal mask for the diagonal score chunk: within chunk
    kt == 2g (global key block 4g+hk), key row k masks query column q iff
    k + 128*hk > q; the same inequality covers the kt == 2g+1 half chunk on
    its first 256 columns.  Applied POST-exp as a 0/1 multiply."""
    k_idx = np.arange(P)[:, None] + P * hk
    q_idx = np.arange(QG)[None, :]
    return np.where(k_idx > q_idx, 0.0, 1.0)


def kernel(
    inputs_for_keys,
    inputs_for_values,
    inputs_for_queries,
    WK,
    WV,
    WQ,
    _trace=False,
):
    import ml_dtypes

    F16N = np.float16
    xk = np.asarray(inputs_for_keys, dtype=np.float32)
    xv = np.asarray(inputs_for_values, dtype=np.float32)
    xq = np.asarray(inputs_for_queries, dtype=np.float32)
    # wv host layout == sbuf layout [p][k][dout]
    wv_h = np.ascontiguousarray(
        np.asarray(WV, np.float32).reshape(KO, P, DOUT)
        .transpose(1, 0, 2).reshape(P, -1)
    ).astype(F16N)
    wq = np.asarray(WQ, dtype=np.float32)
    wk = np.asarray(WK, dtype=np.float32)
    # fused score weight: S = Xq (WQ WK^T / sqrt(d)) Xk^T;  mhT = (WQ WK^T).T
    mh_f = ((wk @ wq.T) * np.float32(1.0 / np.sqrt(DOUT))).astype(np.float32)
    # swizzle so each dout-slice is one contiguous line per partition:
    # mh4[p, t*1024 + k*128 + ii] = Mh.T[k*128+p, t*128+ii]
    mhT = np.ascontiguousarray(
        mh_f.reshape(KO, P, KO, P).transpose(1, 2, 0, 3).reshape(P, -1)
    ).astype(F16N)

    masks = {
        hk: _mask_tile(hk).astype(ml_dtypes.bfloat16) for hk in (0, 1)
    }
    kidx = {hk: _key_index(hk) for hk in (0, 1)}
    xqTb = [np.ascontiguousarray(xq[b].T).astype(F16N) for b in range(B)]

    in_maps = []
    for i in range(8):
        b, hk = i // 2, i % 2
        in_maps.append(
            {
                "xqT": xqTb[b],
                "xkT": np.ascontiguousarray(xk[b][kidx[hk]].T).astype(F16N),
                "xvT": np.ascontiguousarray(xv[b][kidx[hk]].T).astype(F16N),
                "mhT": mhT,
                "wv": wv_h,
                "maskT": masks[hk],
            }
        )

    nc = _get_nc()
    res = run_bass_kernel_spmd(nc, in_maps, list(range(8)), trace=_trace)

    out = np.empty((B, S, DOUT), dtype=np.float32)
    for b in range(B):
        r0 = res.results[2 * b]
        r1 = res.results[2 * b + 1]
        den = (
            np.asarray(r0["l"], np.float32) + np.asarray(r1["l"], np.float32)
        ).reshape(S, 1)
        o01 = np.asarray(r0["ohat"], np.float32) + np.asarray(
            r1["ohat"], np.float32
        )
        out[b] = o01 / den
    if _trace:
        return out, res
    return out


# revision 13
# speedup vs baseline: 1.2202x; 1.2202x over previous
"""Trainium2 Bass kernel for single-head causal attention.

Transposed-softmax layout (S^T, no PE transposes, no max-subtraction),
interleaved-key causal skip, host flash-combine, host-side weight
fusion Mh = WK @ WQ^T / sqrt(d), and a fully-resident fp16/bf16 SBUF
plan:

  score path (Xq, Mh, Xk, K~) in fp16  -- 1.0 PE cycle/row like f32r,
      but half the DMA bytes / SBUF footprint and FWL weight loads
  softmax slab, V, ohat in bf16        -- exp() needs bf16's exponent
      range (scores reach ~+/-50, exp up to ~1e22)
  all matmul accumulation in f32 PSUM; host combines
      out = (ohat0 + ohat1) / (l0 + l1) per batch in f32.

Everything is resident in SBUF (~180 KB/partition of 208), so every
input tile is DMA'd exactly once with no write-after-read hazards:
the PE stream never waits on a buffer-reuse semaphore.  Per-core
phases: warmup (bridges DMA cold start + HAM clock ramp) -> K~ =
Mh @ Xk^T -> g3 score block -> V projection -> attention groups
largest-first.
"""

import numpy as np

import concourse.bass as bass
from concourse import bacc
import concourse.mybir as mybir
import concourse.tile as tile
from concourse.bass_utils import run_bass_kernel_spmd

P = 128
B, S, DIN, DOUT = 4, 2048, 1024, 1024
KSH = S // 2        # key rows per core
KO = DIN // P       # 8 contraction sub-tiles
NT = KSH // P       # 8 key tiles per core
QG = 512            # query-group width (psum free dim)
NG = S // QG        # 4 query groups
F32 = mybir.dt.float32
F16 = mybir.dt.float16
BF16 = mybir.dt.bfloat16
WARM = 8            # warmup matmuls (bridge DMA cold start + HAM ramp)

_NC_CACHE = {}


def _build_bass():
    nc = bacc.Bacc()
    xqT = nc.declare_dram_parameter("xqT", [DIN, S], F16, isOutput=False)
    xkT = nc.declare_dram_parameter("xkT", [DIN, KSH], F16, isOutput=False)
    xvT = nc.declare_dram_parameter("xvT", [DIN, KSH], F16, isOutput=False)
    mhT = nc.declare_dram_parameter("mhT", [P, KO * KO * P], F16, isOutput=False)
    wv = nc.declare_dram_parameter("wv", [P, KO * DOUT], F16, isOutput=False)
    maskT = nc.declare_dram_parameter("maskT", [P, QG], BF16, isOutput=False)
    ohat = nc.declare_dram_parameter("ohat", [S, DOUT], BF16, isOutput=True)
    l_out = nc.declare_dram_parameter("l", [1, S], F32, isOutput=True)

    xq3 = xqT[:, :].rearrange("(o p) q -> p o q", p=P)
    xk3 = xkT[:, :].rearrange("(o p) s -> p o s", p=P)
    xv3 = xvT[:, :].rearrange("(o p) s -> p o s", p=P)
    # host pre-swizzled: mh4[p, t, k, ii] = Mh.T[k*128+p, t*128+ii] so each
    # dout-slice load is one contiguous 2KB line per partition
    mh4 = mhT[:, :].rearrange("p (t k x) -> p t k x", t=KO, k=KO)
    # wv host layout IS the sbuf layout [p][k][dout]: straight big-line loads
    wv3 = wv[:, :].rearrange("p (k x) -> p k x", k=KO)

    with tile.TileContext(nc) as tc:
        with tc.tile_pool(name="persist", bufs=1) as pp:
            # ---- PE warmup: keep the tensor engine busy through the DMA
            # cold start so the HAM clock ramp (3.4us busy window) completes
            # early.  The warm tile lives in the persist pool: a scoped pool
            # would be reused by the input tiles below, making their DMAs
            # wait (WAR) for the warmup matmuls.
            with tc.tile_pool(name="ps_w", bufs=1, space="PSUM") as pswarm:
                wsc = pp.tile([P, 512], F16, name="warm_sc")
                nc.vector.memset(wsc, 0.0)
                wps = pswarm.tile([P, 512], F32, name="warm_ps")
                for i in range(WARM):
                    nc.tensor.matmul(
                        wps, lhsT=wsc[:, 0:P], rhs=wsc,
                        start=(i == 0), stop=(i == WARM - 1),
                    )
            # all inputs resident; every tile DMA'd exactly once
            mh_sb = pp.tile([P, KO, DIN], F16, name="mh")
            xk_sb = pp.tile([P, KO, KSH], F16, name="xk")
            xv_sb = pp.tile([P, KO, KSH], F16, name="xv")
            xq_sb = pp.tile([P, KO, S], F16, name="xq")
            wv_sb = pp.tile([P, KO, DOUT], F16, name="wv")
            kqt_sb = pp.tile([P, KO, KSH], F16, name="kqt")
            v_sb = pp.tile([P, NT, DOUT], BF16, name="v")
            slab_first = pp.tile([P, NT, QG], BF16, name="expT_first")
            m0_sb = pp.tile([P, QG], BF16, name="mask0")
            zeros_sb = pp.tile([P, QG // 2], BF16, name="zeros")
            ones_sb = pp.tile([P, 1], BF16, name="ones")
            l_sb = pp.tile([1, S], F32, name="l_row")
            nc.vector.memset(zeros_sb, 0.0)
            nc.vector.memset(ones_sb, 1.0)

            # ---- DMA issue order == first-use order.  All targets are
            # fresh resident tiles, so no descriptor ever waits on compute.
            nc.sync.dma_start(out=mh_sb[:, :, 0:P], in_=mh4[:, 0])
            for o in range(0, KO, 2):   # K~ chunk 0 feed, o-pair granularity
                nc.sync.dma_start(
                    out=xk_sb[:, o : o + 2, 0:QG], in_=xk3[:, o : o + 2, 0:QG]
                )
            for s in range(1, KO):      # rest of Mh, slice-wise
                nc.sync.dma_start(
                    out=mh_sb[:, :, s * P : (s + 1) * P], in_=mh4[:, s]
                )
            for o in range(0, KO, 2):   # K~ chunk 1 feed
                nc.sync.dma_start(
                    out=xk_sb[:, o : o + 2, QG:KSH], in_=xk3[:, o : o + 2, QG:KSH]
                )
            g3 = (NG - 1) * QG          # g3 score block inputs
            for o in range(0, KO, 4):
                nc.sync.dma_start(
                    out=xq_sb[:, o : o + 4, g3 : g3 + QG],
                    in_=xq3[:, o : o + 4, g3 : g3 + QG],
                )
            nc.sync.dma_start(out=m0_sb, in_=maskT[:, :])
            for k in range(0, KO, 4):   # V-phase inputs
                nc.sync.dma_start(
                    out=wv_sb[:, k : k + 4, :], in_=wv3[:, k : k + 4, :]
                )
            for o in range(0, KO, 4):
                nc.sync.dma_start(
                    out=xv_sb[:, o : o + 4, :], in_=xv3[:, o : o + 4, :]
                )
            for g in (2, 1, 0):         # remaining query groups, use order
                for o in range(0, KO, 4):
                    nc.sync.dma_start(
                        out=xq_sb[:, o : o + 4, g * QG : (g + 1) * QG],
                        in_=xq3[:, o : o + 4, g * QG : (g + 1) * QG],
                    )

            # ---- Phase K~: K~^T = Mh @ Xk^T  [din, keys]
            with tc.tile_pool(name="ps_k", bufs=4, space="PSUM") as psK:
                for c in range(2):
                    for o in range(KO):
                        ps = psK.tile([P, QG], F32, name="kq_ps")
                        for k in range(KO):
                            nc.tensor.matmul(
                                ps,
                                lhsT=mh_sb[:, k, o * P : (o + 1) * P],
                                rhs=xk_sb[:, k, c * QG : (c + 1) * QG],
                                start=(k == 0),
                                stop=(k == KO - 1),
                            )
                        nc.vector.tensor_copy(
                            kqt_sb[:, o, c * QG : (c + 1) * QG], ps
                        )

            with (
                tc.tile_pool(name="exp_tmp", bufs=2) as epool,
                tc.tile_pool(name="slab", bufs=2) as slabpool,
                tc.tile_pool(name="ao", bufs=3) as aopool,
                tc.tile_pool(name="ps_s", bufs=3, space="PSUM") as psS,
                tc.tile_pool(name="ps_l", bufs=1, space="PSUM") as psL,
            ):
                H = QG // 2

                def score_chunk(slab, g, kt):
                    """Scores+exp for one (group, k-tile) [128, 512] chunk of
                    S^T.  kt == 2g+1's first 256 queries are fully masked for
                    both cores (interleaved-key geometry): zero-fill and
                    compute only the second half.  The causal mask pattern is
                    group-independent, so one resident m0 tile serves every
                    diagonal chunk; masking is a post-exp 0/1 multiply so
                    bf16 rounding never touches raw logits."""
                    q0 = g * QG
                    if kt == 2 * g + 1:
                        ps = psS.tile([P, QG], F32, name="score_ps")
                        ph = ps[:, H:]
                        for io in range(KO):
                            nc.tensor.matmul(
                                ph,
                                lhsT=kqt_sb[:, io, kt * P : (kt + 1) * P],
                                rhs=xq_sb[:, io, q0 + H : q0 + QG],
                                start=(io == 0),
                                stop=(io == KO - 1),
                            )
                        nc.vector.tensor_copy(slab[:, kt, :H], zeros_sb)
                        et = epool.tile([P, QG], BF16, name="exp_tmp")
                        nc.scalar.activation(
                            et[:, :H], ph, mybir.ActivationFunctionType.Exp
                        )
                        nc.vector.tensor_tensor(
                            slab[:, kt, H:], et[:, :H], m0_sb[:, :H],
                            mybir.AluOpType.mult,
                        )
                        return
                    ps = psS.tile([P, QG], F32, name="score_ps")
                    for io in range(KO):
                        nc.tensor.matmul(
                            ps,
                            lhsT=kqt_sb[:, io, kt * P : (kt + 1) * P],
                            rhs=xq_sb[:, io, q0 : q0 + QG],
                            start=(io == 0),
                            stop=(io == KO - 1),
                        )
                    if kt == 2 * g:
                        et = epool.tile([P, QG], BF16, name="exp_tmp")
                        nc.scalar.activation(
                            et, ps, mybir.ActivationFunctionType.Exp
                        )
                        nc.vector.tensor_tensor(
                            slab[:, kt, :], et, m0_sb, mybir.AluOpType.mult
                        )
                    else:
                        nc.scalar.activation(
                            slab[:, kt, :], ps, mybir.ActivationFunctionType.Exp
                        )

                # ---- g3 score block: needs only kqt + xq_g3, runs while the
                # V-phase inputs finish streaming
                for kt in range(NT):
                    score_chunk(slab_first, NG - 1, kt)

                # ---- Phase V: V = Xv @ Wv for this core's key blocks
                with tc.tile_pool(name="ps_v", bufs=4, space="PSUM") as psV:
                    for t in range(NT):
                        for dh in range(2):
                            ps = psV.tile([P, QG], F32, name="v_ps")
                            for k in range(KO):
                                nc.tensor.matmul(
                                    ps,
                                    lhsT=xv_sb[:, k, t * P : (t + 1) * P],
                                    rhs=wv_sb[:, k, dh * QG : (dh + 1) * QG],
                                    start=(k == 0),
                                    stop=(k == KO - 1),
                                )
                            nc.vector.tensor_copy(
                                v_sb[:, t, dh * QG : (dh + 1) * QG], ps
                            )

                # ---- Phase A: causal-skip transposed-softmax attention,
                # largest group first
                psO_cm = tc.tile_pool(name="ps_o", bufs=4, space="PSUM")
                psO = psO_cm.__enter__()
                for g in reversed(range(NG)):
                    lim = min(NT, 2 * g + 2)   # k-tiles actually attended
                    if g == NG - 1:
                        slab = slab_first
                    else:
                        slab = slabpool.tile([P, NT, QG], BF16, name="expT")
                        for kt in range(lim):
                            score_chunk(slab, g, kt)

                    ps_l = psL.tile([1, QG], F32, name="l_ps")
                    for kt in range(lim):
                        nc.tensor.matmul(
                            ps_l,
                            lhsT=ones_sb,
                            rhs=slab[:, kt, :],
                            start=(kt == 0),
                            stop=(kt == lim - 1),
                        )
                    nc.vector.tensor_copy(l_sb[:, g * QG : (g + 1) * QG], ps_l)
                    nc.sync.dma_start(
                        out=l_out[:, g * QG : (g + 1) * QG],
                        in_=l_sb[:, g * QG : (g + 1) * QG],
                    )

                    for t in range(QG // P):
                        # first 256 queries of the group can't see the last
                        # (fully masked) key tile
                        kts = range(lim - 1) if t < 2 else range(lim)
                        o_sb = aopool.tile([P, DOUT], BF16, name="attn_out")
                        q0 = g * QG + t * P
                        for dh in range(2):
                            ps = psO.tile([P, QG], F32, name="out_ps")
                            for kt in kts:
                                nc.tensor.matmul(
                                    ps,
                                    lhsT=slab[:, kt, t * P : (t + 1) * P],
                                    rhs=v_sb[:, kt, dh * QG : (dh + 1) * QG],
                                    start=(kt == kts[0]),
                                    stop=(kt == kts[-1]),
                                )
                            if dh == 0:
                                nc.scalar.copy(o_sb[:, :QG], ps)
                            else:
                                nc.vector.tensor_copy(o_sb[:, QG:], ps)
                        # stores ride the sync queue (idle after the input
                        # loads).  In the last group the scalar engine's exp
                        # work is done, so alternate queues there: descriptor
                        # generation is ~0.6us serial per queue and would
                        # otherwise back up the tail after the last matmul.
                        # (Scalar stores mid-stream would block the exp
                        # pipeline behind store semaphores and starve the PE.)
                        eng = nc.scalar if (g == 0 and t % 2 == 1) else nc.sync
                        eng.dma_start(out=ohat[q0 : q0 + P, :], in_=o_sb)
                psO_cm.__exit__(None, None, None)
    nc.finalize()
    return nc


def _get_nc():
    if "nc" not in _NC_CACHE:
        _NC_CACHE["nc"] = _build_bass()
    return _NC_CACHE["nc"]


def _key_index(hk):
    """Global key rows owned by core hk: interleaved 128-row blocks."""
    blocks = np.arange(hk, S // P, 2)
    return (blocks[:, None] * P + np.arange(P)[None, :]).reshape(-1)


def _mask_tile(hk):
    """Multiplicative causal mask for the diagonal score chunk: within chunk
    kt == 2g (global key block 4g+hk), key row k masks query column q iff
    k + 128*hk > q; the same inequality covers the kt == 2g+1 half chunk on
    its first 256 columns.  Applied POST-exp as a 0/1 multiply."""
    k_idx = np.arange(P)[:, None] + P * hk
    q_idx = np.arange(QG)[None, :]
    return np.where(k_idx > q_idx, 0.0, 1.0)


def kernel(
    inputs_for_keys,
    inputs_for_values,
    inputs_for_queries,
    WK,
    WV,
    WQ,
    _trace=False,
):
    import ml_dtypes

    F16N = np.float16
    xk = np.asarray(inputs_for_keys, dtype=np.float32)
    xv = np.asarray(inputs_for_values, dtype=np.float32)
    xq = np.asarray(inputs_for_queries, dtype=np.float32)
    # wv host layout == sbuf layout [p][k][dout]
    wv_h = np.ascontiguousarray(
        np.asarray(WV, np.float32).reshape(KO, P, DOUT)
        .transpose(1, 0, 2).reshape(P, -1)
    ).astype(F16N)
    wq = np.asarray(WQ, dtype=np.float32)
    wk = np.asarray(WK, dtype=np.float32)
    # fused score weight: S = Xq (WQ WK^T / sqrt(d)) Xk^T;  mhT = (WQ WK^T).T
    mh_f = ((wk @ wq.T) * np.float32(1.0 / np.sqrt(DOUT))).astype(np.float32)
    # swizzle so each dout-slice is one contiguous line per partition:
    # mh4[p, t*1024 + k*128 + ii] = Mh.T[k*128+p, t*128+ii]
    mhT = np.ascontiguousarray(
        mh_f.reshape(KO, P, KO, P).transpose(1, 2, 0, 3).reshape(P, -1)
    ).astype(F16N)

    masks = {
        hk: _mask_tile(hk).astype(ml_dtypes.bfloat16) for hk in (0, 1)
    }
    kidx = {hk: _key_index(hk) for hk in (0, 1)}
    xqTb = [np.ascontiguousarray(xq[b].T).astype(F16N) for b in range(B)]

    in_maps = []
    for i in range(8):
        b, hk = i // 2, i % 2
        in_maps.append(
            {
                "xqT": xqTb[b],
                "xkT": np.ascontiguousarray(xk[b][kidx[hk]].T).astype(F16N),
                "xvT": np.ascontiguousarray(xv[b][kidx[hk]].T).astype(F16N),
                "mhT": mhT,
                "wv": wv_h,
                "maskT": masks[hk],
            }
        )

    nc = _get_nc()
    res = run_bass_kernel_spmd(nc, in_maps, list(range(8)), trace=_trace)

    out = np.empty((B, S, DOUT), dtype=np.float32)
    for b in range(B):
        r0 = res.results[2 * b]
        r1 = res.results[2 * b + 1]
        den = (
            np.asarray(r0["l"], np.float32) + np.asarray(r1["l"], np.float32)
        ).reshape(S, 1)
        o01 = np.asarray(r0["ohat"], np.float32) + np.asarray(
            r1["ohat"], np.float32
        )
        out[b] = o01 / den
    if _trace:
        return out, res
    return out


# revision 15
# speedup vs baseline: 1.2263x; 1.0051x over previous
"""Trainium2 Bass kernel for single-head causal attention.

Transposed-softmax layout (S^T, no PE transposes, no max-subtraction),
interleaved-key causal skip, host flash-combine, host-side weight
fusion Mh = WK @ WQ^T / sqrt(d), and a fully-resident fp16/bf16 SBUF
plan:

  score path (Xq, Mh, Xk, K~) in fp16  -- 1.0 PE cycle/row like f32r,
      but half the DMA bytes / SBUF footprint and FWL weight loads
  softmax slab, V, ohat in bf16        -- exp() needs bf16's exponent
      range (scores reach ~+/-50, exp up to ~1e22)
  all matmul accumulation in f32 PSUM; host combines
      out = (ohat0 + ohat1) / (l0 + l1) per batch in f32.

Everything is resident in SBUF (~180 KB/partition of 208), so every
input tile is DMA'd exactly once with no write-after-read hazards:
the PE stream never waits on a buffer-reuse semaphore.  Per-core
phases: warmup (bridges DMA cold start + HAM clock ramp) -> K~ =
Mh @ Xk^T -> g3 score block -> V projection -> attention groups
largest-first.
"""

import numpy as np

import concourse.bass as bass
from concourse import bacc
import concourse.mybir as mybir
import concourse.tile as tile
from concourse.bass_utils import run_bass_kernel_spmd

P = 128
B, S, DIN, DOUT = 4, 2048, 1024, 1024
KSH = S // 2        # key rows per core
KO = DIN // P       # 8 contraction sub-tiles
NT = KSH // P       # 8 key tiles per core
QG = 512            # query-group width (psum free dim)
NG = S // QG        # 4 query groups
F32 = mybir.dt.float32
F16 = mybir.dt.float16
BF16 = mybir.dt.bfloat16
WARM = 9            # warmup matmuls (bridge DMA cold start + HAM ramp)

_NC_CACHE = {}


def _build_bass():
    nc = bacc.Bacc()
    xqT = nc.declare_dram_parameter("xqT", [DIN, S], F16, isOutput=False)
    xkT = nc.declare_dram_parameter("xkT", [DIN, KSH], F16, isOutput=False)
    xvT = nc.declare_dram_parameter("xvT", [DIN, KSH], F16, isOutput=False)
    mhT = nc.declare_dram_parameter("mhT", [P, KO * KO * P], F16, isOutput=False)
    wv = nc.declare_dram_parameter("wv", [P, KO * DOUT], F16, isOutput=False)
    maskT = nc.declare_dram_parameter("maskT", [P, QG], BF16, isOutput=False)
    ohat = nc.declare_dram_parameter("ohat", [S, DOUT], BF16, isOutput=True)
    l_out = nc.declare_dram_parameter("l", [1, S], F32, isOutput=True)

    xq3 = xqT[:, :].rearrange("(o p) q -> p o q", p=P)
    xk3 = xkT[:, :].rearrange("(o p) s -> p o s", p=P)
    xv3 = xvT[:, :].rearrange("(o p) s -> p o s", p=P)
    # host pre-swizzled: mh4[p, t, k, ii] = Mh.T[k*128+p, t*128+ii] so each
    # dout-slice load is one contiguous 2KB line per partition
    mh4 = mhT[:, :].rearrange("p (t k x) -> p t k x", t=KO, k=KO)
    # wv host layout IS the sbuf layout [p][k][dout]: straight big-line loads
    wv3 = wv[:, :].rearrange("p (k x) -> p k x", k=KO)

    with tile.TileContext(nc) as tc:
        with tc.tile_pool(name="persist", bufs=1) as pp:
            # ---- PE warmup: keep the tensor engine busy through the DMA
            # cold start so the HAM clock ramp (3.4us busy window) completes
            # early.  The warm tile lives in the persist pool: a scoped pool
            # would be reused by the input tiles below, making their DMAs
            # wait (WAR) for the warmup matmuls.
            with tc.tile_pool(name="ps_w", bufs=1, space="PSUM") as pswarm:
                wsc = pp.tile([P, 512], F16, name="warm_sc")
                nc.vector.memset(wsc, 0.0)
                wps = pswarm.tile([P, 512], F32, name="warm_ps")
                for i in range(WARM):
                    nc.tensor.matmul(
                        wps, lhsT=wsc[:, 0:P], rhs=wsc,
                        start=(i == 0), stop=(i == WARM - 1),
                    )
            # all inputs resident; every tile DMA'd exactly once
            mh_sb = pp.tile([P, KO, DIN], F16, name="mh")
            xk_sb = pp.tile([P, KO, KSH], F16, name="xk")
            xv_sb = pp.tile([P, KO, KSH], F16, name="xv")
            xq_sb = pp.tile([P, KO, S], F16, name="xq")
            wv_sb = pp.tile([P, KO, DOUT], F16, name="wv")
            kqt_sb = pp.tile([P, KO, KSH], F16, name="kqt")
            v_sb = pp.tile([P, NT, DOUT], BF16, name="v")
            slab_first = pp.tile([P, NT, QG], BF16, name="expT_first")
            m0_sb = pp.tile([P, QG], BF16, name="mask0")
            zeros_sb = pp.tile([P, QG // 2], BF16, name="zeros")
            ones_sb = pp.tile([P, 1], BF16, name="ones")
            l_sb = pp.tile([1, S], F32, name="l_row")
            nc.vector.memset(zeros_sb, 0.0)
            nc.vector.memset(ones_sb, 1.0)

            # ---- DMA issue order == first-use order.  All targets are
            # fresh resident tiles, so no descriptor ever waits on compute.
            # K~ chunk-0 feed split across the sync and scalar queues so all
            # descriptors are issued by ~8.5us (one queue takes ~0.7us per
            # descriptor-gen, serially).  The whole chunk-0 feed is needed
            # within ~1.5us of the K~ start: every o-group sweeps all 8
            # contraction tiles.
            nc.sync.dma_start(out=mh_sb[:, :, 0:P], in_=mh4[:, 0])
            nc.scalar.dma_start(out=xk_sb[:, 2:4, 0:QG], in_=xk3[:, 2:4, 0:QG])
            nc.sync.dma_start(out=xk_sb[:, 0:2, 0:QG], in_=xk3[:, 0:2, 0:QG])
            nc.scalar.dma_start(out=xk_sb[:, 6:8, 0:QG], in_=xk3[:, 6:8, 0:QG])
            nc.sync.dma_start(out=xk_sb[:, 4:6, 0:QG], in_=xk3[:, 4:6, 0:QG])
            for s in range(1, KO):      # rest of Mh, slice-wise
                nc.sync.dma_start(
                    out=mh_sb[:, :, s * P : (s + 1) * P], in_=mh4[:, s]
                )
            for o in range(0, KO, 2):   # K~ chunk 1 feed
                nc.sync.dma_start(
                    out=xk_sb[:, o : o + 2, QG:KSH], in_=xk3[:, o : o + 2, QG:KSH]
                )
            g3 = (NG - 1) * QG          # g3 score block inputs
            for o in range(0, KO, 4):
                nc.sync.dma_start(
                    out=xq_sb[:, o : o + 4, g3 : g3 + QG],
                    in_=xq3[:, o : o + 4, g3 : g3 + QG],
                )
            nc.sync.dma_start(out=m0_sb, in_=maskT[:, :])
            for k in range(0, KO, 4):   # V-phase inputs
                nc.sync.dma_start(
                    out=wv_sb[:, k : k + 4, :], in_=wv3[:, k : k + 4, :]
                )
            for o in range(0, KO, 4):
                nc.sync.dma_start(
                    out=xv_sb[:, o : o + 4, :], in_=xv3[:, o : o + 4, :]
                )
            for g in (2, 1, 0):         # remaining query groups, use order
                for o in range(0, KO, 4):
                    nc.sync.dma_start(
                        out=xq_sb[:, o : o + 4, g * QG : (g + 1) * QG],
                        in_=xq3[:, o : o + 4, g * QG : (g + 1) * QG],
                    )

            # ---- Phase K~: K~^T = Mh @ Xk^T  [din, keys]
            with tc.tile_pool(name="ps_k", bufs=4, space="PSUM") as psK:
                for c in range(2):
                    for o in range(KO):
                        ps = psK.tile([P, QG], F32, name="kq_ps")
                        for k in range(KO):
                            nc.tensor.matmul(
                                ps,
                                lhsT=mh_sb[:, k, o * P : (o + 1) * P],
                                rhs=xk_sb[:, k, c * QG : (c + 1) * QG],
                                start=(k == 0),
                                stop=(k == KO - 1),
                            )
                        nc.vector.tensor_copy(
                            kqt_sb[:, o, c * QG : (c + 1) * QG], ps
                        )

            with (
                tc.tile_pool(name="exp_tmp", bufs=2) as epool,
                tc.tile_pool(name="slab", bufs=2) as slabpool,
                tc.tile_pool(name="ao", bufs=3) as aopool,
                tc.tile_pool(name="ps_s", bufs=3, space="PSUM") as psS,
                tc.tile_pool(name="ps_l", bufs=1, space="PSUM") as psL,
            ):
                H = QG // 2

                def score_chunk(slab, g, kt):
                    """Scores+exp for one (group, k-tile) [128, 512] chunk of
                    S^T.  kt == 2g+1's first 256 queries are fully masked for
                    both cores (interleaved-key geometry): zero-fill and
                    compute only the second half.  The causal mask pattern is
                    group-independent, so one resident m0 tile serves every
                    diagonal chunk; masking is a post-exp 0/1 multiply so
                    bf16 rounding never touches raw logits."""
                    q0 = g * QG
                    if kt == 2 * g + 1:
                        ps = psS.tile([P, QG], F32, name="score_ps")
                        ph = ps[:, H:]
                        for io in range(KO):
                            nc.tensor.matmul(
                                ph,
                                lhsT=kqt_sb[:, io, kt * P : (kt + 1) * P],
                                rhs=xq_sb[:, io, q0 + H : q0 + QG],
                                start=(io == 0),
                                stop=(io == KO - 1),
                            )
                        nc.vector.tensor_copy(slab[:, kt, :H], zeros_sb)
                        et = epool.tile([P, QG], BF16, name="exp_tmp")
                        nc.scalar.activation(
                            et[:, :H], ph, mybir.ActivationFunctionType.Exp
                        )
                        nc.vector.tensor_tensor(
                            slab[:, kt, H:], et[:, :H], m0_sb[:, :H],
                            mybir.AluOpType.mult,
                        )
                        return
                    ps = psS.tile([P, QG], F32, name="score_ps")
                    for io in range(KO):
                        nc.tensor.matmul(
                            ps,
                            lhsT=kqt_sb[:, io, kt * P : (kt + 1) * P],
                            rhs=xq_sb[:, io, q0 : q0 + QG],
                            start=(io == 0),
                            stop=(io == KO - 1),
                        )
                    if kt == 2 * g:
                        et = epool.tile([P, QG], BF16, name="exp_tmp")
                        nc.scalar.activation(
                            et, ps, mybir.ActivationFunctionType.Exp
                        )
                        nc.vector.tensor_tensor(
                            slab[:, kt, :], et, m0_sb, mybir.AluOpType.mult
                        )
                    else:
                        nc.scalar.activation(
                            slab[:, kt, :], ps, mybir.ActivationFunctionType.Exp
                        )

                # ---- g3 score block: needs only kqt + xq_g3, runs while the
                # V-phase inputs finish streaming
                for kt in range(NT):
                    score_chunk(slab_first, NG - 1, kt)

                # ---- Phase V: V = Xv @ Wv for this core's key blocks
                with tc.tile_pool(name="ps_v", bufs=4, space="PSUM") as psV:
                    for t in range(NT):
                        for dh in range(2):
                            ps = psV.tile([P, QG], F32, name="v_ps")
                            for k in range(KO):
                                nc.tensor.matmul(
                                    ps,
                                    lhsT=xv_sb[:, k, t * P : (t + 1) * P],
                                    rhs=wv_sb[:, k, dh * QG : (dh + 1) * QG],
                                    start=(k == 0),
                                    stop=(k == KO - 1),
                                )
                            nc.vector.tensor_copy(
                                v_sb[:, t, dh * QG : (dh + 1) * QG], ps
                            )

                # ---- Phase A: causal-skip transposed-softmax attention,
                # largest group first
                psO_cm = tc.tile_pool(name="ps_o", bufs=4, space="PSUM")
                psO = psO_cm.__enter__()
                for g in reversed(range(NG)):
                    lim = min(NT, 2 * g + 2)   # k-tiles actually attended
                    if g == NG - 1:
                        slab = slab_first
                    else:
                        slab = slabpool.tile([P, NT, QG], BF16, name="expT")
                        for kt in range(lim):
                            score_chunk(slab, g, kt)

                    ps_l = psL.tile([1, QG], F32, name="l_ps")
                    for kt in range(lim):
                        nc.tensor.matmul(
                            ps_l,
                            lhsT=ones_sb,
                            rhs=slab[:, kt, :],
                            start=(kt == 0),
                            stop=(kt == lim - 1),
                        )
                    nc.vector.tensor_copy(l_sb[:, g * QG : (g + 1) * QG], ps_l)
                    nc.sync.dma_start(
                        out=l_out[:, g * QG : (g + 1) * QG],
                        in_=l_sb[:, g * QG : (g + 1) * QG],
                    )

                    for t in range(QG // P):
                        # first 256 queries of the group can't see the last
                        # (fully masked) key tile
                        kts = range(lim - 1) if t < 2 else range(lim)
                        o_sb = aopool.tile([P, DOUT], BF16, name="attn_out")
                        q0 = g * QG + t * P
                        for dh in range(2):
                            ps = psO.tile([P, QG], F32, name="out_ps")
                            for kt in kts:
                                nc.tensor.matmul(
                                    ps,
                                    lhsT=slab[:, kt, t * P : (t + 1) * P],
                                    rhs=v_sb[:, kt, dh * QG : (dh + 1) * QG],
                                    start=(kt == kts[0]),
                                    stop=(kt == kts[-1]),
                                )
                            if dh == 0:
                                nc.scalar.copy(o_sb[:, :QG], ps)
                            else:
                                nc.vector.tensor_copy(o_sb[:, QG:], ps)
                        # stores ride the sync queue (idle after the input
                        # loads).  In the last group the scalar engine's exp
                        # work is done, so alternate queues there: descriptor
                        # generation is ~0.6us serial per queue and would
                        # otherwise back up the tail after the last matmul.
                        # (Scalar stores mid-stream would block the exp
                        # pipeline behind store semaphores and starve the PE.)
                        eng = nc.scalar if (g == 0 and t % 2 == 1) else nc.sync
                        eng.dma_start(out=ohat[q0 : q0 + P, :], in_=o_sb)
                psO_cm.__exit__(None, None, None)
    nc.finalize()
    return nc


def _get_nc():
    if "nc" not in _NC_CACHE:
        _NC_CACHE["nc"] = _build_bass()
    return _NC_CACHE["nc"]


def _key_index(hk):
    """Global key rows owned by core hk: interleaved 128-row blocks."""
    blocks = np.arange(hk, S // P, 2)
    return (blocks[:, None] * P + np.arange(P)[None, :]).reshape(-1)


def _mask_tile(hk):
    """Multiplicative causal mask for the diagonal score chunk: within chunk
    kt == 2g (global key block 4g+hk), key row k masks query column q iff
    k + 128*hk > q; the same inequality covers the kt == 2g+1 half chunk on
    its first 256 columns.  Applied POST-exp as a 0/1 multiply."""
    k_idx = np.arange(P)[:, None] + P * hk
    q_idx = np.arange(QG)[None, :]
    return np.where(k_idx > q_idx, 0.0, 1.0)


def kernel(
    inputs_for_keys,
    inputs_for_values,
    inputs_for_queries,
    WK,
    WV,
    WQ,
    _trace=False,
):
    import ml_dtypes

    F16N = np.float16
    xk = np.asarray(inputs_for_keys, dtype=np.float32)
    xv = np.asarray(inputs_for_values, dtype=np.float32)
    xq = np.asarray(inputs_for_queries, dtype=np.float32)
    # wv host layout == sbuf layout [p][k][dout]
    wv_h = np.ascontiguousarray(
        np.asarray(WV, np.float32).reshape(KO, P, DOUT)
        .transpose(1, 0, 2).reshape(P, -1)
    ).astype(F16N)
    wq = np.asarray(WQ, dtype=np.float32)
    wk = np.asarray(WK, dtype=np.float32)
    # fused score weight: S = Xq (WQ WK^T / sqrt(d)) Xk^T;  mhT = (WQ WK^T).T
    mh_f = ((wk @ wq.T) * np.float32(1.0 / np.sqrt(DOUT))).astype(np.float32)
    # swizzle so each dout-slice is one contiguous line per partition:
    # mh4[p, t*1024 + k*128 + ii] = Mh.T[k*128+p, t*128+ii]
    mhT = np.ascontiguousarray(
        mh_f.reshape(KO, P, KO, P).transpose(1, 2, 0, 3).reshape(P, -1)
    ).astype(F16N)

    masks = {
        hk: _mask_tile(hk).astype(ml_dtypes.bfloat16) for hk in (0, 1)
    }
    kidx = {hk: _key_index(hk) for hk in (0, 1)}
    xqTb = [np.ascontiguousarray(xq[b].T).astype(F16N) for b in range(B)]

    in_maps = []
    for i in range(8):
        b, hk = i // 2, i % 2
        in_maps.append(
            {
                "xqT": xqTb[b],
                "xkT": np.ascontiguousarray(xk[b][kidx[hk]].T).astype(F16N),
                "xvT": np.ascontiguousarray(xv[b][kidx[hk]].T).astype(F16N),
                "mhT": mhT,
                "wv": wv_h,
                "maskT": masks[hk],
            }
        )

    nc = _get_nc()
    res = run_bass_kernel_spmd(nc, in_maps, list(range(8)), trace=_trace)

    out = np.empty((B, S, DOUT), dtype=np.float32)
    for b in range(B):
        r0 = res.results[2 * b]
        r1 = res.results[2 * b + 1]
        den = (
            np.asarray(r0["l"], np.float32) + np.asarray(r1["l"], np.float32)
        ).reshape(S, 1)
        o01 = np.asarray(r0["ohat"], np.float32) + np.asarray(
            r1["ohat"], np.float32
        )
        out[b] = o01 / den
    if _trace:
        return out, res
    return out
